# revision 13
# baseline (speedup 1.0000x reference)
"""NonLocalBlock (B=4, C=256, H=W=64) Trainium2 Bass kernel, v2.

Sharding: 8 cores = 4 batch elements x 2 query-row shards of 2048 rows.
Each core receives its batch element's x rotated along N so that its
query rows are columns [0, 2048) -- pure SPMD.

v2 changes vs v1 (217us baseline):
  * fp16 logit path (theta/phi/wo); fp8e4 for the post-softmax paths:
    P (exp output) and g.  The y and r matmuls then run in fp8 DoubleRow
    perf mode (one matmul per 2 key-chunks), halving the PE instruction
    count for the attention-value phase.  Numpy+CoreSim-validated:
    1.67e-2 max rel vs the 2e-2 gate (errors average over the 4096-key
    softmax sum; the logit path stays fp16 because theta/phi in fp8
    measured 2.04e-2).
  * EXP runs on [128, 1024] chunk-pairs (one ScalarE op per 2 chunks),
    writing fp8 directly -- no separate cast pass.
  * rho = 1/r via DVE reciprocal_approx_fast (frees ScalarE, kills the
    Ln/Exp pair per block).
  * Input DMAs via hardware DGE split across the sync and scalar
    sequencers; x arrives in 512-col blocks interleaved with the
    projections and the first attention block so the PE starts early.
  * Dummy 2KB collective at kernel start warms up the ncfw stack (the
    first collective in a NEFF pays a ~55us bring-up); the real BN-stats
    exchange is an AllGather + local DVE reduce (skips the mesh
    AllReduce's post-gather phases).
  * BN apply split ScalarE (a*o+b) + DVE (+x residual, fp16 out); output
    DMA'd as fp16 and upcast on the host.

PSUM budget (8 banks): s pairs 2x[128,1024] (4) + y [128,512] (1) +
r [128,512] (1) + proj/out-conv shared [128,1024] (2).
"""

import math

import numpy as np

import concourse.bass as bass
import concourse.mybir as mybir
import concourse.tile as tile
from concourse import bacc
from concourse.bass_utils import run_bass_kernel_spmd

# Problem constants (hardcoded per contract).
B, C, HGT, WID = 4, 256, 64, 64
N = HGT * WID            # 4096 spatial positions
D = C // 2               # 128 inner channels
P = 128                  # SBUF partitions
NCORES = 8
SPLIT = NCORES // B      # query shards per batch element
NQ = N // SPLIT          # 2048 query rows per core
CB = C // P              # 2 channel chunks
MCH = N // P             # 32 key chunks
NBLK = 512               # query block (one PSUM bank)
NB = NQ // NBLK          # 4 blocks
NPAIR = MCH // 2         # 16 key-chunk pairs per block
EPS = 1e-5
SCALE = 1.0 / math.sqrt(D)
NSAMP = float(B * N)     # BN sample count per channel

F32 = mybir.dt.float32
F16 = mybir.dt.float16
F8 = mybir.dt.float8e4

AF = mybir.ActivationFunctionType
ALU = mybir.AluOpType
AX = mybir.AxisListType
PM = mybir.MatmulPerfMode

_CACHED_NC = None

# cpack column layout: bq | bk | bv_wide(1024) | gamma(2) | beta(2)
CP_BQ = 0
CP_BK = 1
CP_BV = 2
CP_GAM = CP_BV + 1024
CP_BET = CP_GAM + CB
CP_COLS = CP_BET + CB


def _compile_with_joint_act_tables(nc):
    """Run bacc passes with Exp/Ln resolving to the joint table set (avoids
    ~1.3us table reloads when the two alternate)."""
    real = bacc.get_activation_tables

    def patched(arch):
        t = dict(real(arch))
        for k in ("exp_and_others", "natural_log"):
            if k in t:
                t[k] = type(t[k])()
        return t

    bacc.get_activation_tables = patched
    try:
        nc.compile()
    finally:
        bacc.get_activation_tables = real


def _build_nc():
    nc = bacc.Bacc("TRN2", target_bir_lowering=False, debug=False,
                   num_devices=NCORES)

    x_d = nc.dram_tensor("x", [C, N], F32, kind="ExternalInput")
    # fp16 weights: wq_ct | wk_ct (chunk-major transposed) | wo_t
    wp_d = nc.dram_tensor("wpack", [P, 3 * C], F16, kind="ExternalInput")
    wv_d = nc.dram_tensor("wvb", [P, C], F16, kind="ExternalInput")
    cp_d = nc.dram_tensor("cpack", [P, CP_COLS], F32, kind="ExternalInput")
    out_d = nc.dram_tensor("out", [C, NQ], F16, kind="ExternalOutput")

    with tile.TileContext(nc) as tc:
        with (
            tc.tile_pool(name="consts", bufs=1) as consts,
            tc.tile_pool(name="bigs", bufs=1) as bigs,
            tc.tile_pool(name="ptp8", bufs=3) as ptp8,
            tc.tile_pool(name="work", bufs=2) as work,
            tc.tile_pool(name="ps", bufs=1, space="PSUM") as ps,
            tc.tile_pool(name="dram", bufs=1, space="DRAM") as dram,
        ):
            # ---- x load (f32, hw DGE, 512-col blocks) ----
            # cb0 blocks issue from the sync sequencer, cb1 from scalar:
            # two hw-DGE issuers halve the per-DMA issue serialization.
            NXB = N // NBLK  # 8 blocks per channel chunk
            xs = [bigs.tile([P, N], F32, name=f"x{cb}", tag=f"x{cb}")
                  for cb in range(CB)]
            for k in range(2):
                ksl = slice(k * NBLK, (k + 1) * NBLK)
                nc.sync.dma_start(xs[0][:, ksl], x_d[0:P, ksl])
                nc.scalar.dma_start(xs[1][:, ksl], x_d[P:2 * P, ksl])

            # ---- collective warm-up: dummy 2KB AllGather, result unused ----
            # (the first collective in a NEFF pays ~55us of ncfw bring-up;
            # burn it here, overlapped with the attention phase)
            dummy_sb = consts.tile([P, 2 * CB], F32)
            nc.vector.memset(dummy_sb[:], 0.0)
            cc_warm_in = dram.tile([P, 2 * CB], F32)
            cc_warm_out = dram.tile([NCORES * P, 2 * CB], F32)
            nc.sync.dma_start(cc_warm_in[:], dummy_sb[:])
            nc.gpsimd.collective_compute(
                "AllGather", ALU.bypass,
                replica_groups=[list(range(NCORES))],
                ins=[cc_warm_in[:].opt()], outs=[cc_warm_out[:].opt()])

            # ---- constant / weight loads via hw DGE ----
            wpack = consts.tile([P, 3 * C], F16)
            wvb = consts.tile([P, C], F16)
            cpack = consts.tile([P, CP_COLS], F32)
            nc.sync.dma_start(wpack[:], wp_d[:])
            nc.sync.dma_start(wvb[:], wv_d[:])
            nc.sync.dma_start(cpack[:], cp_d[:])
            wq = wpack[:, 0 * C:1 * C]
            wk = wpack[:, 1 * C:2 * C]
            wo = wpack[:, 2 * C:3 * C]
            bq = cpack[:, CP_BQ:CP_BQ + 1]
            bk = cpack[:, CP_BK:CP_BK + 1]
            bv = cpack[:, CP_BV:CP_BV + 1024]
            gam = cpack[:, CP_GAM:CP_GAM + CB]
            bet = cpack[:, CP_BET:CP_BET + CB]
            ones8 = consts.tile([P, C], F8)
            nc.vector.memset(ones8[:], 1.0)
            ones8_dr = ones8[:].rearrange("p (two d) -> p two d", two=2)

            for k in range(2, NXB):
                ksl = slice(k * NBLK, (k + 1) * NBLK)
                nc.sync.dma_start(xs[0][:, ksl], x_d[0:P, ksl])
                nc.scalar.dma_start(xs[1][:, ksl], x_d[P:2 * P, ksl])
            x16 = [bigs.tile([P, N], F16, name=f"xh{cb}", tag=f"xh{cb}")
                   for cb in range(CB)]

            def cast_block(k):  # x f32 -> fp16 on DVE
                ksl = slice(k * NBLK, (k + 1) * NBLK)
                for cb in range(CB):
                    with nc.allow_low_precision("fp16 projection input"):
                        nc.vector.tensor_copy(x16[cb][:, ksl], xs[cb][:, ksl])

            theta = bigs.tile([P, NQ], F16, tag="theta")
            phi = bigs.tile([P, N], F16, tag="phi")
            gT = bigs.tile([P, N], F8, tag="gT")  # [m%128, 128*mc + d]

            def proj_theta(t):  # theta blocks 2t, 2t+1
                pt = ps.tile([P, 1024], F32, tag="po", bufs=1, name="pt_t")
                for h in range(2):
                    jsl = slice((2 * t + h) * NBLK, (2 * t + h + 1) * NBLK)
                    for cb in range(CB):
                        nc.tensor.matmul(
                            pt[:, h * NBLK:(h + 1) * NBLK],
                            wq[:, cb * P:(cb + 1) * P], x16[cb][:, jsl],
                            start=(cb == 0), stop=(cb == CB - 1))
                for h in range(2):
                    jsl = slice((2 * t + h) * NBLK, (2 * t + h + 1) * NBLK)
                    nc.vector.tensor_scalar_add(
                        theta[:, jsl], pt[:, h * NBLK:(h + 1) * NBLK], bq[:])

            def proj_phi(t):  # phi blocks 2t, 2t+1
                pt = ps.tile([P, 1024], F32, tag="po", bufs=1, name="pt_p")
                for h in range(2):
                    ksl = slice((2 * t + h) * NBLK, (2 * t + h + 1) * NBLK)
                    for cb in range(CB):
                        nc.tensor.matmul(
                            pt[:, h * NBLK:(h + 1) * NBLK],
                            wk[:, cb * P:(cb + 1) * P], x16[cb][:, ksl],
                            start=(cb == 0), stop=(cb == CB - 1))
                for h in range(2):
                    ksl = slice((2 * t + h) * NBLK, (2 * t + h + 1) * NBLK)
                    nc.vector.tensor_scalar_add(
                        phi[:, ksl], pt[:, h * NBLK:(h + 1) * NBLK], bk[:])

            def proj_g(t):  # gT chunks 8t .. 8t+7
                pt = ps.tile([P, 1024], F32, tag="po", bufs=1, name="pt_g")
                for q in range(8):
                    msl = slice((8 * t + q) * P, (8 * t + q + 1) * P)
                    for cb in range(CB):
                        nc.tensor.matmul(
                            pt[:, q * P:(q + 1) * P],
                            x16[cb][:, msl], wvb[:, cb * P:(cb + 1) * P],
                            start=(cb == 0), stop=(cb == CB - 1))
                gsl = slice(8 * t * P, 8 * (t + 1) * P)
                with nc.allow_low_precision("fp8 attention values"):
                    nc.vector.tensor_add(gT[:, gsl], pt[:], bv[:])

            # ---- phase B state ----
            outs = [bigs.tile([P, NQ], F32, name=f"out{cb}", tag=f"out{cb}")
                    for cb in range(CB)]
            s1 = consts.tile([P, CB * NB], F32)
            s2 = consts.tile([P, CB * NB], F32)

            def attn_pairs(j, pcs, y_ps, r_ps):
                """S + EXP(fp8) + y/r DoubleRow for pairs `pcs` of block j."""
                jsl = slice(j * NBLK, (j + 1) * NBLK)
                for pc in pcs:
                    s_ps = ps.tile([P, 1024], F32, tag="ps_s", bufs=2,
                                   name="s_ps")
                    for h in range(2):
                        msl = slice((2 * pc + h) * P, (2 * pc + h + 1) * P)
                        nc.tensor.matmul(s_ps[:, h * NBLK:(h + 1) * NBLK],
                                         phi[:, msl], theta[:, jsl],
                                         start=True, stop=True)
                    pT8 = ptp8.tile([P, 1024], F8, tag="pT8", name="pT8")
                    nc.scalar.activation(pT8[:], s_ps[:], AF.Exp, scale=SCALE)
                    pT8_dr = pT8[:].rearrange("p (two n) -> p two n", two=2)
                    gT_dr = gT[:, 2 * pc * P:(2 * pc + 2) * P].rearrange(
                        "p (two d) -> p two d", two=2)
                    nc.tensor.matmul(
                        y_ps[:], gT_dr, pT8_dr,
                        start=(pc == 0), stop=(pc == NPAIR - 1),
                        perf_mode=PM.DoubleRow)
                    nc.tensor.matmul(
                        r_ps[:], ones8_dr, pT8_dr,
                        start=(pc == 0), stop=(pc == NPAIR - 1),
                        perf_mode=PM.DoubleRow)

            def block_tail(j, y_ps, r_ps):
                """rho, out-conv, stats for block j."""
                jsl = slice(j * NBLK, (j + 1) * NBLK)
                rho = work.tile([P, NBLK], F32, tag="rho", name="rho")
                nc.vector.reciprocal_approx_fast(rho[:], r_ps[:])
                ysb = work.tile([P, NBLK], F16, tag="ysb", name="ysb")
                with nc.allow_low_precision("fp16 attention numerator"):
                    nc.vector.tensor_copy(ysb[:], y_ps[:])
                o_ps = ps.tile([P, 1024], F32, tag="po", bufs=1, name="o_ps")
                for cb in range(CB):
                    nc.tensor.matmul(o_ps[:, cb * NBLK:(cb + 1) * NBLK],
                                     wo[:, cb * P:(cb + 1) * P], ysb[:],
                                     start=True, stop=True)
                for cb in range(CB):
                    col = slice(cb * NB + j, cb * NB + j + 1)
                    nc.vector.scalar_tensor_tensor(
                        out=outs[cb][:, jsl],
                        in0=o_ps[:, cb * NBLK:(cb + 1) * NBLK], scalar=1.0,
                        in1=rho[:], op0=ALU.mult, op1=ALU.mult,
                        accum_out=s1[:, col])
                    sq = work.tile([P, NBLK], F32, tag="sq", name="sq")
                    nc.vector.scalar_tensor_tensor(
                        out=sq[:], in0=outs[cb][:, jsl], scalar=1.0,
                        in1=outs[cb][:, jsl], op0=ALU.mult, op1=ALU.mult,
                        accum_out=s2[:, col])

            # ---- phase A/B interleaved schedule ----
            cast_block(0)
            cast_block(1)
            proj_theta(0)
            proj_phi(0)
            proj_g(0)
            cast_block(2)
            cast_block(3)
            proj_phi(1)
            proj_g(1)
            y_ps0 = ps.tile([P, NBLK], F32, tag="ps_y", bufs=1, name="y_ps")
            r_ps0 = ps.tile([P, NBLK], F32, tag="ps_r", bufs=1, name="r_ps")
            attn_pairs(0, list(range(0, 4)), y_ps0, r_ps0)
            cast_block(4)
            cast_block(5)
            proj_theta(1)
            proj_phi(2)
            proj_g(2)
            attn_pairs(0, list(range(4, 8)), y_ps0, r_ps0)
            cast_block(6)
            cast_block(7)
            proj_phi(3)
            proj_g(3)
            attn_pairs(0, list(range(8, NPAIR)), y_ps0, r_ps0)
            block_tail(0, y_ps0, r_ps0)
            for j in range(1, NB):
                y_psj = ps.tile([P, NBLK], F32, tag="ps_y", bufs=1,
                                name="y_ps")
                r_psj = ps.tile([P, NBLK], F32, tag="ps_r", bufs=1,
                                name="r_ps")
                attn_pairs(j, list(range(NPAIR)), y_psj, r_psj)
                block_tail(j, y_psj, r_psj)

            # ---- phase C: BN stats allreduce + apply + residual ----
            stats = consts.tile([P, 2 * CB], F32)
            for cb in range(CB):
                nc.vector.tensor_reduce(
                    stats[:, cb:cb + 1], s1[:, cb * NB:(cb + 1) * NB],
                    axis=AX.X, op=ALU.add)
                nc.vector.tensor_reduce(
                    stats[:, CB + cb:CB + cb + 1], s2[:, cb * NB:(cb + 1) * NB],
                    axis=AX.X, op=ALU.add)

            cc_in = dram.tile([P, 2 * CB], F32)
            cc_out = dram.tile([NCORES * P, 2 * CB], F32)
            nc.sync.dma_start(cc_in[:], stats[:])
            nc.gpsimd.collective_compute(
                "AllGather", ALU.bypass,
                replica_groups=[list(range(NCORES))],
                ins=[cc_in[:].opt()], outs=[cc_out[:].opt()])
            # readback [8, 128, 4] -> sbuf [p, i*4+c], then reduce over i
            gall = consts.tile([P, NCORES * 2 * CB], F32)
            nc.sync.dma_start(
                gall[:].rearrange("p (i c) -> p i c", i=NCORES),
                cc_out[:].rearrange("(i p) c -> p i c", i=NCORES))
            gstats = consts.tile([P, 2 * CB], F32)
            nc.vector.tensor_reduce(
                gstats[:],
                gall[:].rearrange("p (i c) -> p c i", i=NCORES),
                axis=AX.X, op=ALU.add)

            mean = consts.tile([P, CB], F32)
            var = consts.tile([P, CB], F32)
            tmp = consts.tile([P, CB], F32)
            rstd = consts.tile([P, CB], F32)
            a_sc = consts.tile([P, CB], F32)
            b_sc = consts.tile([P, CB], F32)
            nc.vector.tensor_scalar_mul(mean[:], gstats[:, 0:CB], 1.0 / NSAMP)
            nc.vector.tensor_mul(tmp[:], mean[:], mean[:])
            nc.vector.scalar_tensor_tensor(
                out=var[:], in0=gstats[:, CB:2 * CB], scalar=1.0 / NSAMP,
                in1=tmp[:], op0=ALU.mult, op1=ALU.subtract)
            # rstd = exp(-0.5 * ln(var + eps))
            eps_t = consts.tile([P, 1], F32)
            nc.vector.memset(eps_t[:], EPS)
            nc.scalar.activation(tmp[:], var[:], AF.Ln, bias=eps_t[:])
            nc.scalar.activation(rstd[:], tmp[:], AF.Exp, scale=-0.5)
            nc.vector.tensor_mul(a_sc[:], gam[:], rstd[:])
            nc.vector.tensor_mul(tmp[:], a_sc[:], mean[:])
            nc.vector.tensor_sub(b_sc[:], bet[:], tmp[:])

            # apply: t = a*o + b (ScalarE), f = t + x fp16 (DVE), DMA out
            GRP = 1024
            for cb in range(CB):
                for g0 in range(NQ // GRP):
                    gsl = slice(g0 * GRP, (g0 + 1) * GRP)
                    t = work.tile([P, GRP], F32, tag="t_apply", name="t_ap")
                    nc.scalar.activation(t[:], outs[cb][:, gsl], AF.Identity,
                                         bias=b_sc[:, cb:cb + 1],
                                         scale=a_sc[:, cb:cb + 1])
                    f = work.tile([P, GRP], F16, tag="f_apply", name="f_ap")
                    with nc.allow_low_precision("fp16 output"):
                        nc.vector.tensor_add(f[:], t[:], xs[cb][:, gsl])
                    nc.sync.dma_start(out_d[cb * P:(cb + 1) * P, gsl], f[:])

    _compile_with_joint_act_tables(nc)
    return nc


def _get_nc():
    global _CACHED_NC
    if _CACHED_NC is None:
        _CACHED_NC = _build_nc()
    return _CACHED_NC


def _in_maps(inputs):
    x = np.ascontiguousarray(np.asarray(inputs["x"], np.float32)).reshape(B, C, N)
    tw = np.asarray(inputs["theta_w"], np.float32)
    pw = np.asarray(inputs["phi_w"], np.float32)
    gw = np.asarray(inputs["g_w"], np.float32)
    ow = np.asarray(inputs["out_w"], np.float32)

    def pack_ct(w):  # [D, C] -> [128, C] chunk-major transposed
        wt = np.ascontiguousarray(w.T)            # [C, D]
        return np.concatenate([wt[cb * P:(cb + 1) * P, :] for cb in range(CB)],
                              axis=1)             # [P, CB*D]

    wpack = np.concatenate(
        [pack_ct(tw), pack_ct(pw),
         np.ascontiguousarray(ow.T)], axis=1).astype(np.float16)
    wvb = pack_ct(gw).astype(np.float16)
    bq = np.asarray(inputs["theta_b"], np.float32).reshape(P, 1)
    bk = np.asarray(inputs["phi_b"], np.float32).reshape(P, 1)
    bv = np.broadcast_to(np.asarray(inputs["g_b"], np.float32)[None, :], (P, P))
    bv_wide = np.tile(bv, (1, 8))                 # [128, 1024]
    gam = np.asarray(inputs["gamma"], np.float32).reshape(CB, P).T
    bet = np.asarray(inputs["beta"], np.float32).reshape(CB, P).T
    cpack = np.ascontiguousarray(
        np.concatenate([bq, bk, bv_wide, gam, bet], axis=1))

    maps = []
    for core in range(NCORES):
        b, h = divmod(core, SPLIT)
        n0 = h * NQ
        xr = x[b] if n0 == 0 else np.ascontiguousarray(
            np.concatenate([x[b][:, n0:], x[b][:, :n0]], axis=1))
        maps.append({"x": xr, "wpack": wpack, "wvb": wvb, "cpack": cpack})
    return maps


def _run(inputs, trace=False, **kw):
    nc = _get_nc()
    maps = _in_maps(inputs)
    r = run_bass_kernel_spmd(nc, maps, list(range(NCORES)), trace=trace, **kw)
    out = np.empty((B, C, N), np.float32)
    for core in range(NCORES):
        b, h = divmod(core, SPLIT)
        out[b][:, h * NQ:(h + 1) * NQ] = r.results[core]["out"].astype(np.float32)
    return out.reshape(B, C, HGT, WID), r


def kernel(**inputs):
    out, _ = _run(inputs, trace=False)
    return out


# revision 17
# speedup vs baseline: 1.0103x; 1.0103x over previous
"""NonLocalBlock (B=4, C=256, H=W=64) Trainium2 Bass kernel, v2.

Sharding: 8 cores = 4 batch elements x 2 query-row shards of 2048 rows.
Each core receives its batch element's x rotated along N so that its
query rows are columns [0, 2048) -- pure SPMD.

v2 changes vs v1 (217us baseline):
  * fp16 logit path (theta/phi/wo); fp8e4 for the post-softmax paths:
    P (exp output) and g.  The y and r matmuls then run in fp8 DoubleRow
    perf mode (one matmul per 2 key-chunks), halving the PE instruction
    count for the attention-value phase.  Numpy+CoreSim-validated:
    1.67e-2 max rel vs the 2e-2 gate (errors average over the 4096-key
    softmax sum; the logit path stays fp16 because theta/phi in fp8
    measured 2.04e-2).
  * EXP runs on [128, 1024] chunk-pairs (one ScalarE op per 2 chunks),
    writing fp8 directly -- no separate cast pass.
  * rho = 1/r via DVE reciprocal_approx_fast (frees ScalarE, kills the
    Ln/Exp pair per block).
  * Input DMAs via hardware DGE split across the sync and scalar
    sequencers; x arrives in 512-col blocks interleaved with the
    projections and the first attention block so the PE starts early.
  * Dummy 2KB collective at kernel start warms up the ncfw stack (the
    first collective in a NEFF pays a ~55us bring-up); the real BN-stats
    exchange is an AllGather + local DVE reduce (skips the mesh
    AllReduce's post-gather phases).
  * BN apply split ScalarE (a*o+b) + DVE (+x residual, fp16 out); output
    DMA'd as fp16 and upcast on the host.

PSUM budget (8 banks): s pairs 2x[128,1024] (4) + y [128,512] (1) +
r [128,512] (1) + proj/out-conv shared [128,1024] (2).
"""

import math

import numpy as np

import concourse.bass as bass
import concourse.mybir as mybir
import concourse.tile as tile
from concourse import bacc
from concourse.bass_utils import run_bass_kernel_spmd

# Problem constants (hardcoded per contract).
B, C, HGT, WID = 4, 256, 64, 64
N = HGT * WID            # 4096 spatial positions
D = C // 2               # 128 inner channels
P = 128                  # SBUF partitions
NCORES = 8
SPLIT = NCORES // B      # query shards per batch element
NQ = N // SPLIT          # 2048 query rows per core
CB = C // P              # 2 channel chunks
MCH = N // P             # 32 key chunks
NBLK = 512               # query block (one PSUM bank)
NB = NQ // NBLK          # 4 blocks
NPAIR = MCH // 2         # 16 key-chunk pairs per block
EPS = 1e-5
SCALE = 1.0 / math.sqrt(D)
NSAMP = float(B * N)     # BN sample count per channel

F32 = mybir.dt.float32
F16 = mybir.dt.float16
F8 = mybir.dt.float8e4

AF = mybir.ActivationFunctionType
ALU = mybir.AluOpType
AX = mybir.AxisListType
PM = mybir.MatmulPerfMode

_CACHED_NC = None

# cpack column layout: bq | bk | bv_wide(1024) | gamma(2) | beta(2)
CP_BQ = 0
CP_BK = 1
CP_BV = 2
CP_GAM = CP_BV + 1024
CP_BET = CP_GAM + CB
CP_COLS = CP_BET + CB


def _compile_with_joint_act_tables(nc):
    """Run bacc passes with Exp/Ln resolving to the joint table set (avoids
    ~1.3us table reloads when the two alternate)."""
    real = bacc.get_activation_tables

    def patched(arch):
        t = dict(real(arch))
        for k in ("exp_and_others", "natural_log"):
            if k in t:
                t[k] = type(t[k])()
        return t

    bacc.get_activation_tables = patched
    try:
        nc.compile()
    finally:
        bacc.get_activation_tables = real


def _build_nc():
    nc = bacc.Bacc("TRN2", target_bir_lowering=False, debug=False,
                   num_devices=NCORES)

    x_d = nc.dram_tensor("x", [C, N], F32, kind="ExternalInput")
    # fp16 weights: wq_ct | wk_ct (chunk-major transposed) | wo_t
    wp_d = nc.dram_tensor("wpack", [P, 3 * C], F16, kind="ExternalInput")
    wv_d = nc.dram_tensor("wvb", [P, C], F16, kind="ExternalInput")
    cp_d = nc.dram_tensor("cpack", [P, CP_COLS], F32, kind="ExternalInput")
    out_d = nc.dram_tensor("out", [C, NQ], F16, kind="ExternalOutput")

    with tile.TileContext(nc) as tc:
        with (
            tc.tile_pool(name="consts", bufs=1) as consts,
            tc.tile_pool(name="bigs", bufs=1) as bigs,
            tc.tile_pool(name="ptp8", bufs=3) as ptp8,
            tc.tile_pool(name="work", bufs=2) as work,
            tc.tile_pool(name="ps", bufs=1, space="PSUM") as ps,
            tc.tile_pool(name="dram", bufs=1, space="DRAM") as dram,
        ):
            # ---- x load (f32, hw DGE, 512-col blocks) ----
            # cb0 blocks issue from the sync sequencer, cb1 from scalar:
            # two hw-DGE issuers halve the per-DMA issue serialization.
            NXB = N // NBLK  # 8 blocks per channel chunk
            xs = [bigs.tile([P, N], F32, name=f"x{cb}", tag=f"x{cb}")
                  for cb in range(CB)]
            for k in range(2):
                ksl = slice(k * NBLK, (k + 1) * NBLK)
                nc.sync.dma_start(xs[0][:, ksl], x_d[0:P, ksl])
                nc.scalar.dma_start(xs[1][:, ksl], x_d[P:2 * P, ksl])

            # ---- collective warm-up: dummy 2KB AllReduce, result unused ----
            # (the first collective in a NEFF pays ~55us of ncfw bring-up;
            # burn it here, overlapped with the attention phase)
            dummy_sb = consts.tile([P, 2 * CB], F32)
            nc.vector.memset(dummy_sb[:], 0.0)
            cc_warm_in = dram.tile([P, 2 * CB], F32)
            cc_warm_out = dram.tile([P, 2 * CB], F32)
            nc.sync.dma_start(cc_warm_in[:], dummy_sb[:])
            nc.gpsimd.collective_compute(
                "AllReduce", ALU.add,
                replica_groups=[list(range(NCORES))],
                ins=[cc_warm_in[:].opt()], outs=[cc_warm_out[:].opt()])

            # ---- constant / weight loads via hw DGE ----
            wpack = consts.tile([P, 3 * C], F16)
            wvb = consts.tile([P, C], F16)
            cpack = consts.tile([P, CP_COLS], F32)
            nc.sync.dma_start(wpack[:], wp_d[:])
            nc.sync.dma_start(wvb[:], wv_d[:])
            nc.sync.dma_start(cpack[:], cp_d[:])
            wq = wpack[:, 0 * C:1 * C]
            wk = wpack[:, 1 * C:2 * C]
            wo = wpack[:, 2 * C:3 * C]
            bq = cpack[:, CP_BQ:CP_BQ + 1]
            bk = cpack[:, CP_BK:CP_BK + 1]
            bv = cpack[:, CP_BV:CP_BV + 1024]
            gam = cpack[:, CP_GAM:CP_GAM + CB]
            bet = cpack[:, CP_BET:CP_BET + CB]
            ones8 = consts.tile([P, C], F8)
            nc.vector.memset(ones8[:], 1.0)
            ones8_dr = ones8[:].rearrange("p (two d) -> p two d", two=2)

            for k in range(2, NXB):
                ksl = slice(k * NBLK, (k + 1) * NBLK)
                nc.sync.dma_start(xs[0][:, ksl], x_d[0:P, ksl])
                nc.scalar.dma_start(xs[1][:, ksl], x_d[P:2 * P, ksl])
            x16 = [bigs.tile([P, N], F16, name=f"xh{cb}", tag=f"xh{cb}")
                   for cb in range(CB)]

            def cast_block(k):  # x f32 -> fp16 on DVE
                ksl = slice(k * NBLK, (k + 1) * NBLK)
                for cb in range(CB):
                    with nc.allow_low_precision("fp16 projection input"):
                        nc.vector.tensor_copy(x16[cb][:, ksl], xs[cb][:, ksl])

            theta = bigs.tile([P, NQ], F16, tag="theta")
            phi = bigs.tile([P, N], F16, tag="phi")
            gT = bigs.tile([P, N], F8, tag="gT")  # [m%128, 128*mc + d]

            def proj_theta(t):  # theta blocks 2t, 2t+1
                pt = ps.tile([P, 1024], F32, tag="ps_s", bufs=3, name="pt_t")
                for h in range(2):
                    jsl = slice((2 * t + h) * NBLK, (2 * t + h + 1) * NBLK)
                    for cb in range(CB):
                        nc.tensor.matmul(
                            pt[:, h * NBLK:(h + 1) * NBLK],
                            wq[:, cb * P:(cb + 1) * P], x16[cb][:, jsl],
                            start=(cb == 0), stop=(cb == CB - 1))
                for h in range(2):
                    jsl = slice((2 * t + h) * NBLK, (2 * t + h + 1) * NBLK)
                    nc.vector.tensor_scalar_add(
                        theta[:, jsl], pt[:, h * NBLK:(h + 1) * NBLK], bq[:])

            def proj_phi(t):  # phi blocks 2t, 2t+1
                pt = ps.tile([P, 1024], F32, tag="ps_s", bufs=3, name="pt_p")
                for h in range(2):
                    ksl = slice((2 * t + h) * NBLK, (2 * t + h + 1) * NBLK)
                    for cb in range(CB):
                        nc.tensor.matmul(
                            pt[:, h * NBLK:(h + 1) * NBLK],
                            wk[:, cb * P:(cb + 1) * P], x16[cb][:, ksl],
                            start=(cb == 0), stop=(cb == CB - 1))
                for h in range(2):
                    ksl = slice((2 * t + h) * NBLK, (2 * t + h + 1) * NBLK)
                    nc.vector.tensor_scalar_add(
                        phi[:, ksl], pt[:, h * NBLK:(h + 1) * NBLK], bk[:])

            def proj_g(t):  # gT chunks 8t .. 8t+7
                pt = ps.tile([P, 1024], F32, tag="ps_s", bufs=3, name="pt_g")
                for q in range(8):
                    msl = slice((8 * t + q) * P, (8 * t + q + 1) * P)
                    for cb in range(CB):
                        nc.tensor.matmul(
                            pt[:, q * P:(q + 1) * P],
                            x16[cb][:, msl], wvb[:, cb * P:(cb + 1) * P],
                            start=(cb == 0), stop=(cb == CB - 1))
                gsl = slice(8 * t * P, 8 * (t + 1) * P)
                with nc.allow_low_precision("fp8 attention values"):
                    nc.vector.tensor_add(gT[:, gsl], pt[:], bv[:])

            # ---- phase B state ----
            outs = [bigs.tile([P, NQ], F32, name=f"out{cb}", tag=f"out{cb}")
                    for cb in range(CB)]
            s1 = consts.tile([P, CB * NB], F32)
            s2 = consts.tile([P, CB * NB], F32)

            def attn_pairs(j, pcs, yr_ps):
                """S + EXP(fp8) + y/r DoubleRow for pairs `pcs` of block j."""
                jsl = slice(j * NBLK, (j + 1) * NBLK)
                for pc in pcs:
                    s_ps = ps.tile([P, 1024], F32, tag="ps_s", bufs=3,
                                   name="s_ps")
                    for h in range(2):
                        msl = slice((2 * pc + h) * P, (2 * pc + h + 1) * P)
                        nc.tensor.matmul(s_ps[:, h * NBLK:(h + 1) * NBLK],
                                         phi[:, msl], theta[:, jsl],
                                         start=True, stop=True)
                    pT8 = ptp8.tile([P, 1024], F8, tag="pT8", name="pT8")
                    nc.scalar.activation(pT8[:], s_ps[:], AF.Exp, scale=SCALE)
                    pT8_dr = pT8[:].rearrange("p (two n) -> p two n", two=2)
                    gT_dr = gT[:, 2 * pc * P:(2 * pc + 2) * P].rearrange(
                        "p (two d) -> p two d", two=2)
                    nc.tensor.matmul(
                        yr_ps[:, 0:NBLK], gT_dr, pT8_dr,
                        start=(pc == 0), stop=(pc == NPAIR - 1),
                        perf_mode=PM.DoubleRow)
                    nc.tensor.matmul(
                        yr_ps[:, NBLK:2 * NBLK], ones8_dr, pT8_dr,
                        start=(pc == 0), stop=(pc == NPAIR - 1),
                        perf_mode=PM.DoubleRow)

            def block_tail(j, yr_ps):
                """rho, out-conv, stats for block j."""
                jsl = slice(j * NBLK, (j + 1) * NBLK)
                rho = work.tile([P, NBLK], F32, tag="rho", name="rho")
                nc.vector.reciprocal_approx_fast(rho[:], yr_ps[:, NBLK:2 * NBLK])
                ysb = work.tile([P, NBLK], F16, tag="ysb", name="ysb")
                with nc.allow_low_precision("fp16 attention numerator"):
                    nc.vector.tensor_copy(ysb[:], yr_ps[:, 0:NBLK])
                o_ps = ps.tile([P, 1024], F32, tag="ps_s", bufs=3, name="o_ps")
                for cb in range(CB):
                    nc.tensor.matmul(o_ps[:, cb * NBLK:(cb + 1) * NBLK],
                                     wo[:, cb * P:(cb + 1) * P], ysb[:],
                                     start=True, stop=True)
                for cb in range(CB):
                    col = slice(cb * NB + j, cb * NB + j + 1)
                    nc.vector.scalar_tensor_tensor(
                        out=outs[cb][:, jsl],
                        in0=o_ps[:, cb * NBLK:(cb + 1) * NBLK], scalar=1.0,
                        in1=rho[:], op0=ALU.mult, op1=ALU.mult,
                        accum_out=s1[:, col])
                    sq = work.tile([P, NBLK], F32, tag="sq", name="sq")
                    nc.vector.scalar_tensor_tensor(
                        out=sq[:], in0=outs[cb][:, jsl], scalar=1.0,
                        in1=outs[cb][:, jsl], op0=ALU.mult, op1=ALU.mult,
                        accum_out=s2[:, col])

            # ---- phase A/B interleaved schedule ----
            cast_block(0)
            cast_block(1)
            proj_theta(0)
            proj_phi(0)
            proj_g(0)
            cast_block(2)
            cast_block(3)
            proj_phi(1)
            proj_g(1)
            yr_ps0 = ps.tile([P, 1024], F32, tag="ps_yr", bufs=1, name="yr_ps")
            attn_pairs(0, list(range(0, 4)), yr_ps0)
            cast_block(4)
            cast_block(5)
            proj_theta(1)
            proj_phi(2)
            proj_g(2)
            attn_pairs(0, list(range(4, 8)), yr_ps0)
            cast_block(6)
            cast_block(7)
            proj_phi(3)
            proj_g(3)
            attn_pairs(0, list(range(8, NPAIR)), yr_ps0)
            block_tail(0, yr_ps0)
            for j in range(1, NB):
                yr_psj = ps.tile([P, 1024], F32, tag="ps_yr", bufs=1,
                                 name="yr_ps")
                attn_pairs(j, list(range(NPAIR)), yr_psj)
                block_tail(j, yr_psj)

            # ---- phase C: BN stats allreduce + apply + residual ----
            stats = consts.tile([P, 2 * CB], F32)
            for cb in range(CB):
                nc.vector.tensor_reduce(
                    stats[:, cb:cb + 1], s1[:, cb * NB:(cb + 1) * NB],
                    axis=AX.X, op=ALU.add)
                nc.vector.tensor_reduce(
                    stats[:, CB + cb:CB + cb + 1], s2[:, cb * NB:(cb + 1) * NB],
                    axis=AX.X, op=ALU.add)

            cc_in = dram.tile([P, 2 * CB], F32)
            cc_out = dram.tile([P, 2 * CB], F32)
            nc.sync.dma_start(cc_in[:], stats[:])
            nc.gpsimd.collective_compute(
                "AllReduce", ALU.add,
                replica_groups=[list(range(NCORES))],
                ins=[cc_in[:].opt()], outs=[cc_out[:].opt()])
            gstats = consts.tile([P, 2 * CB], F32)
            nc.sync.dma_start(gstats[:], cc_out[:])

            mean = consts.tile([P, CB], F32)
            var = consts.tile([P, CB], F32)
            tmp = consts.tile([P, CB], F32)
            rstd = consts.tile([P, CB], F32)
            a_sc = consts.tile([P, CB], F32)
            b_sc = consts.tile([P, CB], F32)
            nc.vector.tensor_scalar_mul(mean[:], gstats[:, 0:CB], 1.0 / NSAMP)
            nc.vector.tensor_mul(tmp[:], mean[:], mean[:])
            nc.vector.scalar_tensor_tensor(
                out=var[:], in0=gstats[:, CB:2 * CB], scalar=1.0 / NSAMP,
                in1=tmp[:], op0=ALU.mult, op1=ALU.subtract)
            # rstd = exp(-0.5 * ln(var + eps))
            eps_t = consts.tile([P, 1], F32)
            nc.vector.memset(eps_t[:], EPS)
            nc.scalar.activation(tmp[:], var[:], AF.Ln, bias=eps_t[:])
            nc.scalar.activation(rstd[:], tmp[:], AF.Exp, scale=-0.5)
            nc.vector.tensor_mul(a_sc[:], gam[:], rstd[:])
            nc.vector.tensor_mul(tmp[:], a_sc[:], mean[:])
            nc.vector.tensor_sub(b_sc[:], bet[:], tmp[:])

            # apply: t = a*o + b (ScalarE), f = t + x fp16 (DVE), DMA out
            GRP = 1024
            for cb in range(CB):
                for g0 in range(NQ // GRP):
                    gsl = slice(g0 * GRP, (g0 + 1) * GRP)
                    t = work.tile([P, GRP], F32, tag="t_apply", name="t_ap")
                    nc.scalar.activation(t[:], outs[cb][:, gsl], AF.Identity,
                                         bias=b_sc[:, cb:cb + 1],
                                         scale=a_sc[:, cb:cb + 1])
                    f = work.tile([P, GRP], F16, tag="f_apply", name="f_ap")
                    with nc.allow_low_precision("fp16 output"):
                        nc.vector.tensor_add(f[:], t[:], xs[cb][:, gsl])
                    nc.sync.dma_start(out_d[cb * P:(cb + 1) * P, gsl], f[:])

    _compile_with_joint_act_tables(nc)
    return nc


def _get_nc():
    global _CACHED_NC
    if _CACHED_NC is None:
        _CACHED_NC = _build_nc()
    return _CACHED_NC


def _in_maps(inputs):
    x = np.ascontiguousarray(np.asarray(inputs["x"], np.float32)).reshape(B, C, N)
    tw = np.asarray(inputs["theta_w"], np.float32)
    pw = np.asarray(inputs["phi_w"], np.float32)
    gw = np.asarray(inputs["g_w"], np.float32)
    ow = np.asarray(inputs["out_w"], np.float32)

    def pack_ct(w):  # [D, C] -> [128, C] chunk-major transposed
        wt = np.ascontiguousarray(w.T)            # [C, D]
        return np.concatenate([wt[cb * P:(cb + 1) * P, :] for cb in range(CB)],
                              axis=1)             # [P, CB*D]

    wpack = np.concatenate(
        [pack_ct(tw), pack_ct(pw),
         np.ascontiguousarray(ow.T)], axis=1).astype(np.float16)
    wvb = pack_ct(gw).astype(np.float16)
    bq = np.asarray(inputs["theta_b"], np.float32).reshape(P, 1)
    bk = np.asarray(inputs["phi_b"], np.float32).reshape(P, 1)
    bv = np.broadcast_to(np.asarray(inputs["g_b"], np.float32)[None, :], (P, P))
    bv_wide = np.tile(bv, (1, 8))                 # [128, 1024]
    gam = np.asarray(inputs["gamma"], np.float32).reshape(CB, P).T
    bet = np.asarray(inputs["beta"], np.float32).reshape(CB, P).T
    cpack = np.ascontiguousarray(
        np.concatenate([bq, bk, bv_wide, gam, bet], axis=1))

    maps = []
    for core in range(NCORES):
        b, h = divmod(core, SPLIT)
        n0 = h * NQ
        xr = x[b] if n0 == 0 else np.ascontiguousarray(
            np.concatenate([x[b][:, n0:], x[b][:, :n0]], axis=1))
        maps.append({"x": xr, "wpack": wpack, "wvb": wvb, "cpack": cpack})
    return maps


def _run(inputs, trace=False, **kw):
    nc = _get_nc()
    maps = _in_maps(inputs)
    r = run_bass_kernel_spmd(nc, maps, list(range(NCORES)), trace=trace, **kw)
    out = np.empty((B, C, N), np.float32)
    for core in range(NCORES):
        b, h = divmod(core, SPLIT)
        out[b][:, h * NQ:(h + 1) * NQ] = r.results[core]["out"].astype(np.float32)
    return out.reshape(B, C, HGT, WID), r


def kernel(**inputs):
    out, _ = _run(inputs, trace=False)
    return out
